# revision 5
# baseline (speedup 1.0000x reference)
"""Distributed Trainium2 kernel for nn_AttentionHead_5214090297398.

Reference computes, with no softmax:
    q = x @ Wq.T + bq; k = x @ Wk.T + bk; v = x @ Wv.T + bv
    out = ((q @ k.T) * sqrt(d)) @ v

By matmul associativity:  out = (q * sqrt(d)) @ (k.T @ v)
where k.T @ v is only [128, 128] — this removes the [8192, 8192]
score matrix entirely.

Sharding: x is row-sharded across 8 cores (1024 rows each). Each core
computes its q, k, v row-blocks, the local partial k_i.T @ v_i, then an
AllGather + on-device reduce yields the full k.T @ v on every core, and
each core finishes its out rows with one small matmul.

Host-side prep (layout/sharding only): x is pre-transposed per shard
(contraction must sit on the SBUF partition axis), weights are
pre-transposed, the sqrt(d) scale is folded into Wq/bq, and everything
is cast to bf16 for TensorEngine full-rate matmuls (f32 runs 4x slower).
"""

import numpy as np
from ml_dtypes import bfloat16

N_CORES = 8
SEQ = 8192
EMB = 1024
D = 128
ROWS = SEQ // N_CORES  # rows of x per core
SCALE = float(np.sqrt(D))

_CACHE: dict = {}


def _build_nc(debug_taps=False):
    import concourse.mybir as mybir
    import concourse.bacc as bacc
    import concourse.tile as tile

    bf = mybir.dt.bfloat16
    f32 = mybir.dt.float32

    nc = bacc.Bacc("TRN2", target_bir_lowering=False, debug=False,
                   num_devices=N_CORES)

    # per-core inputs: xt = x_i.T  [EMB, ROWS]; weights replicated,
    # pre-transposed with the bias as a final row.
    xt = nc.dram_tensor("xt", [EMB, ROWS], bf, kind="ExternalInput").ap()
    wq = nc.dram_tensor("wq", [EMB + 1, D], bf, kind="ExternalInput").ap()
    wkv = nc.dram_tensor("wkv", [EMB + 1, 2 * D], bf,
                         kind="ExternalInput").ap()
    # per-core output: out_i.T  [D, ROWS] (host transposes back)
    out = nc.dram_tensor("out", [D, ROWS], f32, kind="ExternalOutput").ap()

    NE = EMB // 128   # 8 e-chunks
    NT = ROWS // 128  # 8 row-tiles per core
    NH = ROWS // 512  # 2 column-halves of 512

    with tile.TileContext(nc) as tc:
        with (
            tc.tile_pool(name="sb", bufs=1) as sb,
            tc.tile_pool(name="ps", bufs=1, space="PSUM") as ps,
            tc.tile_pool(name="dram", bufs=1, space="DRAM") as dram,
        ):
            # ---- input DMAs ----
            wkv_sb = sb.tile([128, NE, 2 * D], bf, name="wkv_sb")
            nc.sync.dma_start(
                wkv_sb[:],
                wkv[0:EMB, :].rearrange("(c p) d -> p c d", p=128))
            wkv_b = sb.tile([1, 2 * D], bf, name="wkv_b")
            nc.sync.dma_start(wkv_b[:], wkv[EMB:EMB + 1, :])

            ones_sb = sb.tile([1, ROWS], bf, name="ones_sb")
            nc.gpsimd.memset(ones_sb[:], 1.0)

            # x_i.T in 16 tiles of [128, 512]
            xts = []
            for ec in range(NE):
                row = []
                for h in range(NH):
                    t = sb.tile([128, 512], bf, name=f"xts_{ec}_{h}",
                                tag=f"xts_{ec}_{h}")
                    nc.sync.dma_start(
                        t[:], xt[ec * 128:(ec + 1) * 128,
                                 h * 512:(h + 1) * 512])
                    row.append(t)
                xts.append(row)

            wq_sb = sb.tile([128, NE, D], bf, name="wq_sb")
            nc.sync.dma_start(
                wq_sb[:], wq[0:EMB, :].rearrange("(c p) d -> p c d", p=128))
            wq_b = sb.tile([1, D], bf, name="wq_b")
            nc.sync.dma_start(wq_b[:], wq[EMB:EMB + 1, :])

            # ---- phase 1: k,v natural layout [n, d] ----
            # one PSUM bank per row-tile, [128, (k|v)=256]; a bank must
            # hold a single accumulation group.
            psum_kv = [ps.tile([128, 256], f32, name=f"psum_kv{nt}",
                               tag=f"ps_kv{nt}") for nt in range(NT)]
            for ec in range(NE):
                for nt in range(NT):
                    h, c = nt // 4, (nt % 4) * 128
                    nc.tensor.matmul(
                        psum_kv[nt][:],
                        lhsT=xts[ec][h][:, c:c + 128],
                        rhs=wkv_sb[:, ec, :],
                        start=(ec == 0), stop=False)
            for nt in range(NT):  # bias row via K=1 matmul
                nc.tensor.matmul(
                    psum_kv[nt][:],
                    lhsT=ones_sb[:, nt * 128:(nt + 1) * 128],
                    rhs=wkv_b[:],
                    start=False, stop=True)

            # copy k, v to SBUF as bf16 [128, nt, 128]
            k_sb = sb.tile([128, NT, D], bf, name="k_sb")
            v_sb = sb.tile([128, NT, D], bf, name="v_sb")
            for nt in range(NT):
                nc.vector.tensor_copy(k_sb[:, nt, :], psum_kv[nt][:, 0:D])
                nc.vector.tensor_copy(v_sb[:, nt, :], psum_kv[nt][:, D:2 * D])

            # ---- phase 2: partial p = k_i.T @ v_i  [128, 128] ----
            psum_p = ps.tile([128, 512], f32, name="psum_p", tag="ps_kv0")
            for nt in range(NT):
                nc.tensor.matmul(
                    psum_p[:, 0:D], lhsT=k_sb[:, nt, :], rhs=v_sb[:, nt, :],
                    start=(nt == 0), stop=(nt == NT - 1))
            p_sb = sb.tile([128, D], f32, name="p_sb")
            nc.vector.tensor_copy(p_sb[:], psum_p[:, 0:D])

            # ---- phase 3: AllGather partials, reduce on-device ----
            p_bounce = dram.tile([128, D], f32, name="p_bounce")
            ag_out = dram.tile([N_CORES * 128, D], f32, name="ag_out",
                               addr_space="Shared")
            nc.sync.dma_start(p_bounce[:], p_sb[:])
            nc.gpsimd.collective_compute(
                "AllGather",
                mybir.AluOpType.bypass,
                replica_groups=[list(range(N_CORES))],
                ins=[p_bounce.opt()],
                outs=[ag_out.opt()],
            )

            # ---- phase 4 (overlaps AG): q.T = scale*(Wq @ x.T + bq 1^T) ----
            psum_q = [ps.tile([128, 512], f32, name=f"psum_q{h}",
                              tag=f"ps_kv{1 + h}") for h in range(NH)]
            for ec in range(NE):
                for h in range(NH):
                    nc.tensor.matmul(
                        psum_q[h][:], lhsT=wq_sb[:, ec, :],
                        rhs=xts[ec][h][:],
                        start=(ec == 0), stop=False)
            for h in range(NH):
                nc.tensor.matmul(
                    psum_q[h][:], lhsT=wq_b[:],
                    rhs=ones_sb[:, h * 512:(h + 1) * 512],
                    start=False, stop=True)
            qt_sb = sb.tile([128, ROWS], bf, name="qt_sb")
            for h in range(NH):
                nc.vector.tensor_copy(
                    qt_sb[:, h * 512:(h + 1) * 512], psum_q[h][:])

            # ---- phase 5: reduce gathered partials -> ktv [128, 128] ----
            g3 = sb.tile([128, N_CORES, D], f32, name="g3")
            nc.sync.dma_start(
                g3[:], ag_out[:].rearrange("(r p) d -> p r d", p=128))
            acc = sb.tile([128, D], f32, name="acc")
            nc.vector.tensor_add(acc[:], g3[:, 0, :], g3[:, 1, :])
            for r in range(2, N_CORES):
                nc.vector.tensor_add(acc[:], acc[:], g3[:, r, :])
            ktv_sb = sb.tile([128, D], bf, name="ktv_sb")
            nc.vector.tensor_copy(ktv_sb[:], acc[:])

            # ---- phase 6: out.T = ktv.T @ q.T  [128, ROWS] ----
            psum_o = [ps.tile([128, 512], f32, name=f"psum_o{h}",
                              tag=f"ps_kv{3 + h}") for h in range(NH)]
            out_sb = sb.tile([128, ROWS], f32, name="out_sb")
            for h in range(NH):
                nc.tensor.matmul(
                    psum_o[h][:], lhsT=ktv_sb[:],
                    rhs=qt_sb[:, h * 512:(h + 1) * 512],
                    start=True, stop=True)
                nc.vector.tensor_copy(
                    out_sb[:, h * 512:(h + 1) * 512], psum_o[h][:])
                nc.sync.dma_start(
                    out[:, h * 512:(h + 1) * 512],
                    out_sb[:, h * 512:(h + 1) * 512])

            if debug_taps:
                taps = {
                    "dbg_k": (k_sb, [128, NT, D], bf),
                    "dbg_v": (v_sb, [128, NT, D], bf),
                    "dbg_q": (qt_sb, [128, ROWS], bf),
                    "dbg_p": (p_sb, [128, D], f32),
                    "dbg_ktv": (acc, [128, D], f32),
                }
                for name, (t, shape, dt_) in taps.items():
                    ext = nc.dram_tensor(name, shape, dt_,
                                         kind="ExternalOutput").ap()
                    nc.sync.dma_start(ext[:], t[:])

    nc.compile()
    return nc


def _prep_inputs(x, Wq, bq, Wk, bk, Wv, bv):
    s = SCALE
    wq_host = np.concatenate(
        [(Wq.astype(np.float64) * s).T, (bq.astype(np.float64) * s)[None, :]],
        axis=0).astype(bfloat16)
    wkv_host = np.concatenate(
        [np.concatenate([Wk.T, Wv.T], axis=1),
         np.concatenate([bk, bv])[None, :]], axis=0).astype(bfloat16)
    in_maps = []
    for i in range(N_CORES):
        xt_i = np.ascontiguousarray(
            x[i * ROWS:(i + 1) * ROWS, :].T).astype(bfloat16)
        in_maps.append({"xt": xt_i, "wq": wq_host, "wkv": wkv_host})
    return in_maps


def _run(inputs, trace=False, trace_cores=None):
    from concourse.bass_utils import run_bass_kernel_spmd

    if "nc" not in _CACHE:
        _CACHE["nc"] = _build_nc()
    nc = _CACHE["nc"]

    in_maps = _prep_inputs(**inputs)
    res = run_bass_kernel_spmd(nc, in_maps, list(range(N_CORES)),
                               trace=trace, trace_cores=trace_cores)
    blocks = [res.results[i]["out"].T for i in range(N_CORES)]
    full = np.concatenate(blocks, axis=0).astype(np.float32)
    return full, res


def kernel(**inputs) -> np.ndarray:
    out, _ = _run(inputs, trace=False)
    return out


# revision 7
# speedup vs baseline: 1.4197x; 1.4197x over previous
"""Distributed Trainium2 kernel for nn_AttentionHead_5214090297398.

Reference computes, with no softmax:
    q = x @ Wq.T + bq; k = x @ Wk.T + bk; v = x @ Wv.T + bv
    out = ((q @ k.T) * sqrt(d)) @ v

By matmul associativity:  out = (q * sqrt(d)) @ (k.T @ v)
where k.T @ v is only [128, 128] — this removes the [8192, 8192]
score matrix entirely.

Sharding: x is row-sharded across 8 cores (1024 rows each). Each core
computes its q, k, v row-blocks, the local partial k_i.T @ v_i, then an
AllGather + on-device tree-reduce yields the full k.T @ v on every
core, and each core finishes its out rows with one small matmul.

Host-side prep (layout/sharding only): x is pre-transposed per shard
(contraction must sit on the SBUF partition axis), weights are
pre-transposed and pre-swizzled into the SBUF chunk layout so every DMA
descriptor is a contiguous 2-4KB run, the sqrt(d) scale is folded into
Wq/bq, and everything is cast to bf16 for TensorEngine full-rate
matmuls (f32 runs 4x slower).
"""

import numpy as np
from ml_dtypes import bfloat16

N_CORES = 8
SEQ = 8192
EMB = 1024
D = 128
ROWS = SEQ // N_CORES  # rows of x per core
SCALE = float(np.sqrt(D))

_CACHE: dict = {}


def _build_nc(debug_taps=False):
    import concourse.mybir as mybir
    import concourse.bacc as bacc
    import concourse.tile as tile

    bf = mybir.dt.bfloat16
    f32 = mybir.dt.float32

    nc = bacc.Bacc("TRN2", target_bir_lowering=False, debug=False,
                   num_devices=N_CORES)

    NE = EMB // 128   # 8 e-chunks
    NT = ROWS // 128  # 8 row-tiles per core
    NH = ROWS // 512  # 2 column-halves of 512

    # per-core inputs: xt = x_i.T [EMB, ROWS]; weights replicated and
    # pre-swizzled to [128, chunk, d] with the bias row separate.
    xt = nc.dram_tensor("xt", [EMB, ROWS], bf, kind="ExternalInput").ap()
    wq = nc.dram_tensor("wq", [128, NE, D], bf, kind="ExternalInput").ap()
    wqb = nc.dram_tensor("wqb", [1, D], bf, kind="ExternalInput").ap()
    wkv = nc.dram_tensor("wkv", [128, NE, 2 * D], bf,
                         kind="ExternalInput").ap()
    wkvb = nc.dram_tensor("wkvb", [1, 2 * D], bf, kind="ExternalInput").ap()
    # per-core output: out_i.T  [D, ROWS] (host transposes back)
    out = nc.dram_tensor("out", [D, ROWS], f32, kind="ExternalOutput").ap()

    with tile.TileContext(nc) as tc:
        with (
            tc.tile_pool(name="sb", bufs=1) as sb,
            tc.tile_pool(name="ps", bufs=1, space="PSUM") as ps,
            tc.tile_pool(name="dram", bufs=1, space="DRAM") as dram,
        ):
            # ---- input DMAs (contiguous per-partition rows) ----
            wkv_sb = sb.tile([128, NE, 2 * D], bf, name="wkv_sb")
            nc.sync.dma_start(wkv_sb[:], wkv[:])
            wkv_b = sb.tile([1, 2 * D], bf, name="wkv_b")
            nc.sync.dma_start(wkv_b[:], wkvb[:])

            ones_sb = sb.tile([1, ROWS], bf, name="ones_sb")
            nc.gpsimd.memset(ones_sb[:], 1.0)

            # x_i.T in 8 chunks of [128, 1024] (2KB/partition contiguous)
            xts = []
            for ec in range(NE):
                t = sb.tile([128, ROWS], bf, name=f"xts_{ec}",
                            tag=f"xts_{ec}")
                nc.sync.dma_start(t[:], xt[ec * 128:(ec + 1) * 128, :])
                xts.append(t)

            wq_sb = sb.tile([128, NE, D], bf, name="wq_sb")
            nc.sync.dma_start(wq_sb[:], wq[:])
            wq_b = sb.tile([1, D], bf, name="wq_b")
            nc.sync.dma_start(wq_b[:], wqb[:])

            # ---- phase 1: k,v natural layout [n, d] ----
            # one PSUM bank per row-tile ([128, (k|v)=256]); a bank holds
            # a single accumulation group.
            psum_kv = [ps.tile([128, 256], f32, name=f"psum_kv{nt}",
                               tag=f"ps_kv{nt}") for nt in range(NT)]
            for ec in range(NE):
                for nt in range(NT):
                    nc.tensor.matmul(
                        psum_kv[nt][:],
                        lhsT=xts[ec][:, nt * 128:(nt + 1) * 128],
                        rhs=wkv_sb[:, ec, :],
                        start=(ec == 0), stop=False)
            for nt in range(NT):  # bias row via K=1 matmul
                nc.tensor.matmul(
                    psum_kv[nt][:],
                    lhsT=ones_sb[:, nt * 128:(nt + 1) * 128],
                    rhs=wkv_b[:],
                    start=False, stop=True)

            # copy k, v to SBUF as bf16 [128, nt, 128]
            k_sb = sb.tile([128, NT, D], bf, name="k_sb")
            v_sb = sb.tile([128, NT, D], bf, name="v_sb")
            for nt in range(NT):
                nc.vector.tensor_copy(k_sb[:, nt, :], psum_kv[nt][:, 0:D])
                nc.vector.tensor_copy(v_sb[:, nt, :], psum_kv[nt][:, D:2 * D])

            # ---- phase 2: partial p = k_i.T @ v_i  [128, 128] ----
            psum_p = ps.tile([128, 512], f32, name="psum_p", tag="ps_kv0")
            for nt in range(NT):
                nc.tensor.matmul(
                    psum_p[:, 0:D], lhsT=k_sb[:, nt, :], rhs=v_sb[:, nt, :],
                    start=(nt == 0), stop=(nt == NT - 1))
            p_sb = sb.tile([128, D], bf, name="p_sb")
            nc.vector.tensor_copy(p_sb[:], psum_p[:, 0:D])

            # ---- phase 3: AllGather bf16 partials ----
            p_bounce = dram.tile([128, D], bf, name="p_bounce")
            ag_out = dram.tile([N_CORES * 128, D], bf, name="ag_out",
                               addr_space="Shared")
            nc.sync.dma_start(p_bounce[:], p_sb[:])
            nc.gpsimd.collective_compute(
                "AllGather",
                mybir.AluOpType.bypass,
                replica_groups=[list(range(N_CORES))],
                ins=[p_bounce.opt()],
                outs=[ag_out.opt()],
            )

            # ---- phase 4 (overlaps AG): q.T = scale*(Wq @ x.T + bq 1^T) ----
            psum_q = [ps.tile([128, 512], f32, name=f"psum_q{h}",
                              tag=f"ps_kv{1 + h}") for h in range(NH)]
            for ec in range(NE):
                for h in range(NH):
                    nc.tensor.matmul(
                        psum_q[h][:], lhsT=wq_sb[:, ec, :],
                        rhs=xts[ec][:, h * 512:(h + 1) * 512],
                        start=(ec == 0), stop=False)
            for h in range(NH):
                nc.tensor.matmul(
                    psum_q[h][:], lhsT=wq_b[:],
                    rhs=ones_sb[:, h * 512:(h + 1) * 512],
                    start=False, stop=True)
            qt_sb = sb.tile([128, ROWS], bf, name="qt_sb")
            for h in range(NH):
                nc.vector.tensor_copy(
                    qt_sb[:, h * 512:(h + 1) * 512], psum_q[h][:])

            # ---- phase 5: tree-reduce gathered partials -> ktv ----
            g3 = sb.tile([128, N_CORES, D], bf, name="g3")
            nc.sync.dma_start(
                g3[:], ag_out[:].rearrange("(r p) d -> p r d", p=128))
            t4 = sb.tile([128, 4, D], bf, name="t4")
            for j in range(4):
                nc.vector.tensor_add(
                    t4[:, j, :], g3[:, 2 * j, :], g3[:, 2 * j + 1, :])
            t2 = sb.tile([128, 2, D], bf, name="t2")
            for j in range(2):
                nc.vector.tensor_add(
                    t2[:, j, :], t4[:, 2 * j, :], t4[:, 2 * j + 1, :])
            ktv_sb = sb.tile([128, D], bf, name="ktv_sb")
            nc.vector.tensor_add(ktv_sb[:], t2[:, 0, :], t2[:, 1, :])

            # ---- phase 6: out.T = ktv.T @ q.T  [128, ROWS] ----
            psum_o = [ps.tile([128, 512], f32, name=f"psum_o{h}",
                              tag=f"ps_kv{3 + h}") for h in range(NH)]
            out_sb = sb.tile([128, ROWS], f32, name="out_sb")
            for h in range(NH):
                nc.tensor.matmul(
                    psum_o[h][:], lhsT=ktv_sb[:],
                    rhs=qt_sb[:, h * 512:(h + 1) * 512],
                    start=True, stop=True)
                nc.vector.tensor_copy(
                    out_sb[:, h * 512:(h + 1) * 512], psum_o[h][:])
                nc.sync.dma_start(
                    out[:, h * 512:(h + 1) * 512],
                    out_sb[:, h * 512:(h + 1) * 512])

            if debug_taps:
                taps = {
                    "dbg_k": (k_sb, [128, NT, D], bf),
                    "dbg_v": (v_sb, [128, NT, D], bf),
                    "dbg_q": (qt_sb, [128, ROWS], bf),
                    "dbg_p": (p_sb, [128, D], bf),
                    "dbg_ktv": (ktv_sb, [128, D], bf),
                }
                for name, (t, shape, dt_) in taps.items():
                    ext = nc.dram_tensor(name, shape, dt_,
                                         kind="ExternalOutput").ap()
                    nc.sync.dma_start(ext[:], t[:])

    nc.compile()
    return nc


def _prep_inputs(x, Wq, bq, Wk, bk, Wv, bv):
    s = SCALE
    NE = EMB // 128
    # [EMB, d] -> swizzled [128, NE, d] so partition rows are contiguous
    wq_t = (Wq.astype(np.float64) * s).T.astype(bfloat16)
    wq_sw = np.ascontiguousarray(
        wq_t.reshape(NE, 128, D).transpose(1, 0, 2))
    wkv_t = np.concatenate([Wk.T, Wv.T], axis=1).astype(bfloat16)
    wkv_sw = np.ascontiguousarray(
        wkv_t.reshape(NE, 128, 2 * D).transpose(1, 0, 2))
    wqb_h = (bq.astype(np.float64) * s)[None, :].astype(bfloat16)
    wkvb_h = np.concatenate([bk, bv])[None, :].astype(bfloat16)
    in_maps = []
    for i in range(N_CORES):
        xt_i = np.ascontiguousarray(
            x[i * ROWS:(i + 1) * ROWS, :].T).astype(bfloat16)
        in_maps.append({"xt": xt_i, "wq": wq_sw, "wqb": wqb_h,
                        "wkv": wkv_sw, "wkvb": wkvb_h})
    return in_maps


def _run(inputs, trace=False, trace_cores=None):
    from concourse.bass_utils import run_bass_kernel_spmd

    if "nc" not in _CACHE:
        _CACHE["nc"] = _build_nc()
    nc = _CACHE["nc"]

    in_maps = _prep_inputs(**inputs)
    res = run_bass_kernel_spmd(nc, in_maps, list(range(N_CORES)),
                               trace=trace, trace_cores=trace_cores)
    blocks = [res.results[i]["out"].T for i in range(N_CORES)]
    full = np.concatenate(blocks, axis=0).astype(np.float32)
    return full, res


def kernel(**inputs) -> np.ndarray:
    out, _ = _run(inputs, trace=False)
    return out
